# revision 17
# baseline (speedup 1.0000x reference)
"""Trainium2 Bass kernel for retrieval_knn (nn_DSL_19791209300140).

kernel(x, W, b) -> (x, edge_index, edge_attr), matching the reference:
    h = leaky_relu(x @ W + b, 0.01)
    hn = h / ||h||
    sim = hn @ hn.T ; nbr = top8(sim)
    edge_index = [nbr.flat ; repeat(arange(N), 8)]
    edge_attr = mean(x[nbr], axis=1)

Sharding: queries are sharded across 8 NeuronCores. Core i receives
x rotated by -2048*i rows so that its 2048 query rows are local rows
0..2047; keys (all 16384 rows) are recomputed on every core (the FC is
cheap relative to the N^2 similarity). Top-8 indices come back in
rotated-local coordinates and are mapped to global ids on the host.
"""
import os
import sys
from contextlib import ExitStack

import numpy as np

sys.path.insert(0, "/opt/trn_rl_repo")

import concourse.bacc as bacc  # noqa: E402
import concourse.mybir as mybir  # noqa: E402
import concourse.tile as tile  # noqa: E402
from concourse import library_config  # noqa: E402
from concourse.tile_rust import add_dep_helper  # noqa: E402

f32 = mybir.dt.float32
u16 = mybir.dt.uint16
i16 = mybir.dt.int16
AF = mybir.ActivationFunctionType
OP = mybir.AluOpType

P = 128
N_CORES = 8


def build_kernel(N=16384, D=512, D2=256):
    """One SPMD core program: queries = local rows 0..N/8-1, keys = all N."""
    Q = N // N_CORES
    NRG = N // 512  # fc row groups
    NQT = Q // P  # query tiles
    HALF = N // 2
    NKC_H = HALF // 512  # key chunks per half
    DC = D // P  # d chunks (4)

    nc = bacc.Bacc("TRN2", target_bir_lowering=False, debug=False)
    xr = nc.dram_tensor("xr", [N, D], f32, kind="ExternalInput")
    Wt = nc.dram_tensor("W", [D, D2], f32, kind="ExternalInput")
    brow = nc.dram_tensor("b", [P, D2], f32, kind="ExternalInput")
    ident = nc.dram_tensor("ident", [P, P], f32, kind="ExternalInput")
    nbrv = nc.dram_tensor("nbrv", [Q, 8], f32, kind="ExternalOutput")
    nbr16 = nc.dram_tensor("nbr16", [Q, 8], u16, kind="ExternalOutput")
    eattr = nc.dram_tensor("eattr", [Q, D], f32, kind="ExternalOutput")

    with tile.TileContext(nc) as tc, ExitStack() as ctx:
        pers = ctx.enter_context(tc.tile_pool(name="pers", bufs=1))
        hnT_lo = pers.tile([P, N], f32)
        hnT_hi = pers.tile([P, N], f32)

        lib_inst = nc.gpsimd.load_library(library_config.mlp)

        # ---------------- FC phase: hnT[d2, n] for all N rows ----------------
        _skip_fc = bool(int(os.environ.get("KNN_SKIP_FC", "0")))
        _skip_sim = bool(int(os.environ.get("KNN_SKIP_SIM", "0")))
        _skip_gather = bool(int(os.environ.get("KNN_SKIP_GATHER", "0")))
        _topk_level = int(os.environ.get("KNN_TOPK_LEVEL", "3"))
        with ExitStack() as fctx:
            xin = fctx.enter_context(tc.tile_pool(name="xin", bufs=2))
            fcw = fctx.enter_context(tc.tile_pool(name="fcw", bufs=2))
            ps_t = fctx.enter_context(tc.tile_pool(name="ps_t", bufs=2, space="PSUM"))
            ps_h = fctx.enter_context(tc.tile_pool(name="ps_h", bufs=2, space="PSUM"))
            ps_n = fctx.enter_context(tc.tile_pool(name="ps_n", bufs=2, space="PSUM"))
            const = fctx.enter_context(tc.tile_pool(name="const", bufs=1))
            W_t = const.tile([P, DC, D2], f32)
            nc.gpsimd.dma_start(W_t[:], Wt.rearrange("(c p) n -> p c n", p=P))
            b128 = const.tile([P, D2], f32)
            nc.gpsimd.dma_start(b128[:], brow[:])
            id_t = const.tile([P, P], f32)
            nc.gpsimd.dma_start(id_t[:], ident[:])

            for g in range(0 if _skip_fc else NRG):
                x_t = xin.tile([P, 4, D], f32, tag="x")
                nc.sync.dma_start(
                    x_t[:],
                    xr[g * 512 : (g + 1) * 512, :].rearrange("(s p) d -> p s d", p=P),
                )
                xT_t = fcw.tile([P, DC, 512], f32, tag="xT")
                for c in range(DC):
                    pt = ps_t.tile([P, 512], f32, tag="pt")
                    for s in range(4):
                        nc.tensor.transpose(
                            pt[:, s * P : (s + 1) * P],
                            x_t[:, s, c * P : (c + 1) * P],
                            id_t[:],
                        )
                    nc.scalar.activation(xT_t[:, c, :], pt[:], AF.Copy)

                pn_lo = ps_n.tile([P, 512], f32, tag="pn_lo")
                pn_hi = ps_n.tile([P, 512], f32, tag="pn_hi")
                for s in range(4):
                    ph = ps_h.tile([P, D2], f32, tag="ph")
                    for c in range(DC):
                        nc.tensor.matmul(
                            ph[:],
                            xT_t[:, c, s * P : (s + 1) * P],
                            W_t[:, c, :],
                            start=(c == 0),
                            stop=(c == DC - 1),
                        )
                    hcp = fcw.tile([P, D2], f32, tag="hcp")
                    nc.vector.tensor_tensor(hcp[:], ph[:], b128[:], op=OP.add)
                    hl = fcw.tile([P, D2], f32, tag="hl")
                    nc.vector.scalar_tensor_tensor(
                        hl[:], hcp[:], 0.01, hcp[:], OP.mult, OP.max
                    )
                    sq = fcw.tile([P, D2], f32, tag="sq")
                    ss = fcw.tile([P, 1], f32, tag="ss")
                    nc.vector.scalar_tensor_tensor(
                        sq[:], hl[:], 1.0, hl[:], OP.mult, OP.mult, accum_out=ss[:]
                    )
                    sn = fcw.tile([P, 1], f32, tag="sn")
                    nc.scalar.activation(sn[:], ss[:], AF.Sqrt)
                    rn = fcw.tile([P, 1], f32, tag="rn")
                    nc.vector.reciprocal(rn[:], sn[:])
                    hn = fcw.tile([P, D2], f32, tag="hn")
                    nc.scalar.activation(hn[:], hl[:], AF.Copy, scale=rn[:])
                    nc.tensor.transpose(
                        pn_lo[:, s * P : (s + 1) * P], hn[:, 0:P], id_t[:]
                    )
                    nc.tensor.transpose(
                        pn_hi[:, s * P : (s + 1) * P], hn[:, P : 2 * P], id_t[:]
                    )
                nc.scalar.activation(
                    hnT_lo[:, g * 512 : (g + 1) * 512], pn_lo[:], AF.Copy
                )
                nc.scalar.activation(
                    hnT_hi[:, g * 512 : (g + 1) * 512], pn_hi[:], AF.Copy
                )

        # ---------------- sim + topk + gather phase ----------------
        with ExitStack() as sctx:
            simp = sctx.enter_context(tc.tile_pool(name="simp", bufs=2))
            ps_s = sctx.enter_context(tc.tile_pool(name="ps_s", bufs=6, space="PSUM"))
            kw = sctx.enter_context(tc.tile_pool(name="kw", bufs=2))
            gth = sctx.enter_context(tc.tile_pool(name="gth", bufs=1))

            for t in range(0 if _skip_sim else NQT):
                qs = slice(t * P, (t + 1) * P)
                m01 = kw.tile([P, 16], f32, tag="m01")
                iloc = []
                for h in range(2):
                    sh = simp.tile([P, HALF], f32, tag="sh")
                    for kc in range(NKC_H):
                        kslice = slice(h * HALF + kc * 512, h * HALF + (kc + 1) * 512)
                        pp = ps_s.tile([P, 512], f32, tag="pp")
                        nc.tensor.matmul(
                            pp[:], hnT_lo[:, qs], hnT_lo[:, kslice],
                            start=True, stop=False,
                        )
                        nc.tensor.matmul(
                            pp[:], hnT_hi[:, qs], hnT_hi[:, kslice],
                            start=False, stop=True,
                        )
                        nc.scalar.activation(
                            sh[:, kc * 512 : (kc + 1) * 512], pp[:], AF.Copy
                        )
                    # local top-8 + local indices: frees this half's buffer early
                    if _topk_level >= 1:
                        nc.vector.max(m01[:, h * 8 : h * 8 + 8], sh[:])
                    il = kw.tile([P, 8], u16, tag=f"i{h}", name=f"il{h}")
                    if _topk_level >= 2:
                        nc.vector.max_index(il[:], m01[:, h * 8 : h * 8 + 8], sh[:])
                    else:
                        nc.vector.memset(il[:], 0)
                    iloc.append(il)

                # merge by value; resolve indices via a 16-slot position scan
                if _topk_level < 1:
                    nc.vector.memset(m01[:], 0)
                v8 = kw.tile([P, 8], f32, tag="v8")
                nc.vector.max(v8[:], m01[:])
                pos = kw.tile([P, 8], u16, tag="pos")
                nc.vector.max_index(pos[:], v8[:], m01[:])
                ic = kw.tile([P, 16], f32, tag="ic")
                nc.vector.tensor_copy(ic[:, 0:8], iloc[0][:])
                nc.vector.tensor_scalar(
                    ic[:, 8:16], iloc[1][:], float(HALF), None, op0=OP.add
                )
                posf = kw.tile([P, 8], f32, tag="posf")
                nc.vector.tensor_copy(posf[:], pos[:])
                fin = kw.tile([P, 8], f32, tag="fin")
                nc.vector.tensor_copy(fin[:], ic[:, 0:1].to_broadcast([P, 8]))
                mks = kw.tile([P, 8], mybir.dt.uint32, tag="mks")
                for s in range(1, 16):
                    nc.vector.tensor_scalar(
                        mks[:], posf[:], float(s), None, op0=OP.is_equal
                    )
                    nc.vector.copy_predicated(
                        fin[:], mks[:], ic[:, s : s + 1].to_broadcast([P, 8])
                    )
                dma_fin = nc.sync.dma_start(nbrv[qs, :], fin[:])
                fcl = kw.tile([P, 8], f32, tag="fcl")
                nc.vector.tensor_scalar_min(fcl[:], fin[:], float(N - 1))
                fu = kw.tile([P, 8], u16, tag="fu")
                nc.vector.tensor_copy(fu[:], fcl[:])
                dma_u = nc.sync.dma_start(
                    nbr16[qs, :], fu[:]
                )

                if _skip_gather:
                    continue
                # wrapped idx layout for dma_gather: [16, 64] replicated x8
                ix_t = gth.tile([P, 64], u16, tag="ix")
                src = nbr16[qs, :].rearrange("(qh p) k -> p k qh", p=16)
                for gblk in range(8):
                    d = nc.sync.dma_start(ix_t[gblk * 16 : (gblk + 1) * 16, :], src)
                    add_dep_helper(d.ins, dma_u.ins, reason="idx wrap after nbr16 write")

                rr = []
                for gb in range(2):
                    g4 = gth.tile([P, 4, D], f32, tag="g4", name=f"g4_{gb}")
                    gi = nc.gpsimd.dma_gather(
                        g4[:], xr[:],
                        ix_t[:, gb * 32 : (gb + 1) * 32].bitcast(i16),
                        P * 4, P * 4, D,
                    )
                    add_dep_helper(gi.ins, lib_inst.ins, reason="mlp lib before gather")
                    r_t = gth.tile([P, D], f32, tag=f"rt{gb}", name=f"rt{gb}")
                    nc.vector.tensor_reduce(
                        r_t[:],
                        g4[:].rearrange("p k j -> p j k"),
                        axis=mybir.AxisListType.X,
                        op=OP.add,
                    )
                    rr.append(r_t)
                ea = gth.tile([P, D], f32, tag="ea")
                nc.vector.tensor_tensor(ea[:], rr[0][:], rr[1][:], op=OP.add)
                nc.scalar.activation(rr[0][:], ea[:], AF.Copy, scale=0.125)
                nc.sync.dma_start(eattr[qs, :], rr[0][:])

    nc.compile()
    return nc


_CACHED = {}


def _get_kernel(N, D, D2):
    key = (N, D, D2)
    if key not in _CACHED:
        _CACHED[key] = build_kernel(N, D, D2)
    return _CACHED[key]


def _host_fallback_rows(x, W, b, rows, N, K=8):
    """Exact fp32 recompute of top-8 for pathological (tie) rows."""
    h = x.astype(np.float32) @ W.astype(np.float32) + b.astype(np.float32)
    h = np.where(h >= 0, h, np.float32(0.01) * h)
    hn = h / np.maximum(
        np.linalg.norm(h, axis=1, keepdims=True), np.float32(1e-12)
    ).astype(np.float32)
    sim = hn[rows] @ hn.T
    part = np.argpartition(-sim, K, axis=1)[:, : K + 4]
    vals = np.take_along_axis(sim, part, axis=1)
    order = np.argsort(-vals, axis=1, kind="stable")
    return np.take_along_axis(part, order, axis=1)[:, :K]


def kernel(x, W, b):
    x = np.ascontiguousarray(np.asarray(x, dtype=np.float32))
    W = np.ascontiguousarray(np.asarray(W, dtype=np.float32))
    b = np.ascontiguousarray(np.asarray(b, dtype=np.float32))
    N, D = x.shape
    D2 = W.shape[1]
    Q = N // N_CORES
    K = 8

    from concourse.bass_utils import run_bass_kernel_spmd

    nc = _get_kernel(N, D, D2)
    ident = np.eye(P, dtype=np.float32)
    brow = np.ascontiguousarray(np.tile(b.reshape(1, D2), (P, 1)))
    in_maps = []
    for i in range(N_CORES):
        in_maps.append(
            {
                "xr": np.roll(x, -Q * i, axis=0),
                "W": W,
                "b": brow,
                "ident": ident,
            }
        )
    import time as _time

    t0 = _time.time()
    res = run_bass_kernel_spmd(nc, in_maps, core_ids=list(range(N_CORES)))
    kernel.last_exec_wall_s = _time.time() - t0

    nbr = np.empty((N, K), dtype=np.int64)
    edge_attr = np.empty((N, D), dtype=np.float32)
    bad_rows = []
    for i in range(N_CORES):
        r = res.results[i]
        loc = r["nbrv"].astype(np.int64)  # local (rotated) ids, sentinel >= N
        bad = np.unique(np.argwhere(loc >= N)[:, 0])
        glob = (loc + Q * i) % N
        nbr[Q * i : Q * (i + 1)] = glob
        edge_attr[Q * i : Q * (i + 1)] = r["eattr"]
        for rr in bad:
            bad_rows.append(Q * i + int(rr))

    if bad_rows:
        rows = np.array(sorted(set(bad_rows)), dtype=np.int64)
        fixed = _host_fallback_rows(x, W, b, rows, N, K)
        nbr[rows] = fixed
        edge_attr[rows] = x[fixed].mean(axis=1)

    row = nbr.reshape(-1).astype(np.int32)
    col = np.repeat(np.arange(N, dtype=np.int32), K)
    edge_index = np.stack([row, col], axis=0)
    return (x, edge_index, edge_attr)


# revision 19
# speedup vs baseline: 1.1405x; 1.1405x over previous
"""Trainium2 Bass kernel for retrieval_knn (nn_DSL_19791209300140).

kernel(x, W, b) -> (x, edge_index, edge_attr), matching the reference:
    h = leaky_relu(x @ W + b, 0.01)
    hn = h / ||h||
    sim = hn @ hn.T ; nbr = top8(sim)
    edge_index = [nbr.flat ; repeat(arange(N), 8)]
    edge_attr = mean(x[nbr], axis=1)

Sharding: queries are sharded across 8 NeuronCores. Core i receives
x rotated by -2048*i rows so that its 2048 query rows are local rows
0..2047; keys (all 16384 rows) are recomputed on every core (the FC is
cheap relative to the N^2 similarity). Top-8 indices come back in
rotated-local coordinates and are mapped to global ids on the host.
"""
import os
import sys
from contextlib import ExitStack

import numpy as np

sys.path.insert(0, "/opt/trn_rl_repo")

import concourse.bacc as bacc  # noqa: E402
import concourse.mybir as mybir  # noqa: E402
import concourse.tile as tile  # noqa: E402
from concourse import library_config  # noqa: E402
from concourse.tile_rust import add_dep_helper  # noqa: E402

f32 = mybir.dt.float32
u16 = mybir.dt.uint16
i16 = mybir.dt.int16
AF = mybir.ActivationFunctionType
OP = mybir.AluOpType

P = 128
N_CORES = 8


def build_kernel(N=16384, D=512, D2=256):
    """One SPMD core program: queries = local rows 0..N/8-1, keys = all N."""
    Q = N // N_CORES
    NRG = N // 512  # fc row groups
    NQT = Q // P  # query tiles
    HALF = N // 2
    NKC_H = HALF // 512  # key chunks per half
    DC = D // P  # d chunks (4)

    nc = bacc.Bacc("TRN2", target_bir_lowering=False, debug=False)
    xr = nc.dram_tensor("xr", [N, D], f32, kind="ExternalInput")
    Wt = nc.dram_tensor("W", [D, D2], f32, kind="ExternalInput")
    brow = nc.dram_tensor("b", [P, D2], f32, kind="ExternalInput")
    ident = nc.dram_tensor("ident", [P, P], f32, kind="ExternalInput")
    nbrv = nc.dram_tensor("nbrv", [Q, 8], f32, kind="ExternalOutput")
    nbr16 = nc.dram_tensor("nbr16", [Q, 8], u16, kind="ExternalOutput")
    eattr = nc.dram_tensor("eattr", [Q, D], f32, kind="ExternalOutput")

    with tile.TileContext(nc) as tc, ExitStack() as ctx:
        pers = ctx.enter_context(tc.tile_pool(name="pers", bufs=1))
        f16 = mybir.dt.float16
        hnT_alo = pers.tile([P, N], f16)
        hnT_ahi = pers.tile([P, N], f16)
        hnT_rlo = pers.tile([P, N], f16)
        hnT_rhi = pers.tile([P, N], f16)

        lib_inst = nc.gpsimd.load_library(library_config.mlp)

        # ---------------- FC phase: hnT[d2, n] for all N rows ----------------
        _skip_fc = bool(int(os.environ.get("KNN_SKIP_FC", "0")))
        _skip_sim = bool(int(os.environ.get("KNN_SKIP_SIM", "0")))
        _skip_gather = bool(int(os.environ.get("KNN_SKIP_GATHER", "0")))
        _topk_level = int(os.environ.get("KNN_TOPK_LEVEL", "3"))
        with ExitStack() as fctx:
            xin = fctx.enter_context(tc.tile_pool(name="xin", bufs=2))
            fcw = fctx.enter_context(tc.tile_pool(name="fcw", bufs=2))
            ps_t = fctx.enter_context(tc.tile_pool(name="ps_t", bufs=2, space="PSUM"))
            ps_h = fctx.enter_context(tc.tile_pool(name="ps_h", bufs=2, space="PSUM"))
            ps_n = fctx.enter_context(tc.tile_pool(name="ps_n", bufs=1, space="PSUM"))
            const = fctx.enter_context(tc.tile_pool(name="const", bufs=1))
            W_t = const.tile([P, DC, D2], f32)
            nc.gpsimd.dma_start(W_t[:], Wt.rearrange("(c p) n -> p c n", p=P))
            b128 = const.tile([P, D2], f32)
            nc.gpsimd.dma_start(b128[:], brow[:])
            id_t = const.tile([P, P], f32)
            nc.gpsimd.dma_start(id_t[:], ident[:])
            id16 = const.tile([P, P], f16)
            nc.vector.tensor_copy(id16[:], id_t[:])

            for g in range(0 if _skip_fc else NRG):
                x_t = xin.tile([P, 4, D], f32, tag="x")
                nc.sync.dma_start(
                    x_t[:],
                    xr[g * 512 : (g + 1) * 512, :].rearrange("(s p) d -> p s d", p=P),
                )
                xT_t = fcw.tile([P, DC, 512], f32, tag="xT")
                for c in range(DC):
                    pt = ps_t.tile([P, 512], f32, tag="pt")
                    for s in range(4):
                        nc.tensor.transpose(
                            pt[:, s * P : (s + 1) * P],
                            x_t[:, s, c * P : (c + 1) * P],
                            id_t[:],
                        )
                    nc.scalar.activation(xT_t[:, c, :], pt[:], AF.Copy)

                pn = {
                    k: ps_n.tile([P, 512], f16, tag=f"pn_{k}", name=f"pn_{k}")
                    for k in ("alo", "ahi", "rlo", "rhi")
                }
                for s in range(4):
                    ph = ps_h.tile([P, D2], f32, tag="ph")
                    for c in range(DC):
                        nc.tensor.matmul(
                            ph[:],
                            xT_t[:, c, s * P : (s + 1) * P],
                            W_t[:, c, :],
                            start=(c == 0),
                            stop=(c == DC - 1),
                        )
                    hcp = fcw.tile([P, D2], f32, tag="hcp")
                    nc.vector.tensor_tensor(hcp[:], ph[:], b128[:], op=OP.add)
                    hl = fcw.tile([P, D2], f32, tag="hl")
                    nc.vector.scalar_tensor_tensor(
                        hl[:], hcp[:], 0.01, hcp[:], OP.mult, OP.max
                    )
                    sq = fcw.tile([P, D2], f32, tag="sq")
                    ss = fcw.tile([P, 1], f32, tag="ss")
                    nc.vector.scalar_tensor_tensor(
                        sq[:], hl[:], 1.0, hl[:], OP.mult, OP.mult, accum_out=ss[:]
                    )
                    sn = fcw.tile([P, 1], f32, tag="sn")
                    nc.scalar.activation(sn[:], ss[:], AF.Sqrt)
                    rn = fcw.tile([P, 1], f32, tag="rn")
                    nc.vector.reciprocal(rn[:], sn[:])
                    rn2 = fcw.tile([P, 1], f32, tag="rn2")
                    nc.vector.tensor_scalar_mul(rn2[:], rn[:], 2048.0)
                    hn = fcw.tile([P, D2], f32, tag="hn")
                    nc.scalar.activation(hn[:], hl[:], AF.Copy, scale=rn2[:])
                    av = fcw.tile([P, D2], f16, tag="av")
                    nc.vector.tensor_copy(av[:], hn[:])
                    rv = fcw.tile([P, D2], f16, tag="rv")
                    nc.vector.tensor_tensor(rv[:], hn[:], av[:], op=OP.subtract)
                    for src_t, klo, khi in ((av, "alo", "ahi"), (rv, "rlo", "rhi")):
                        nc.tensor.transpose(
                            pn[klo][:, s * P : (s + 1) * P], src_t[:, 0:P], id16[:]
                        )
                        nc.tensor.transpose(
                            pn[khi][:, s * P : (s + 1) * P], src_t[:, P : 2 * P], id16[:]
                        )
                for k, dst in (
                    ("alo", hnT_alo), ("ahi", hnT_ahi),
                    ("rlo", hnT_rlo), ("rhi", hnT_rhi),
                ):
                    nc.scalar.activation(
                        dst[:, g * 512 : (g + 1) * 512], pn[k][:], AF.Copy
                    )

        # ---------------- sim + topk + gather phase ----------------
        with ExitStack() as sctx:
            simp = sctx.enter_context(tc.tile_pool(name="simp", bufs=2))
            ps_s = sctx.enter_context(tc.tile_pool(name="ps_s", bufs=6, space="PSUM"))
            kw = sctx.enter_context(tc.tile_pool(name="kw", bufs=2))
            gth = sctx.enter_context(tc.tile_pool(name="gth", bufs=1))

            for t in range(0 if _skip_sim else NQT):
                qs = slice(t * P, (t + 1) * P)
                m01 = kw.tile([P, 16], f32, tag="m01")
                iloc = []
                for h in range(2):
                    sh = simp.tile([P, HALF], f32, tag="sh")
                    for kc in range(NKC_H):
                        kslice = slice(h * HALF + kc * 512, h * HALF + (kc + 1) * 512)
                        pp = ps_s.tile([P, 512], f32, tag="pp")
                        terms = (
                            (hnT_alo, hnT_rlo), (hnT_ahi, hnT_rhi),
                            (hnT_rlo, hnT_alo), (hnT_rhi, hnT_ahi),
                            (hnT_alo, hnT_alo), (hnT_ahi, hnT_ahi),
                        )
                        for ti, (qt, kt) in enumerate(terms):
                            nc.tensor.matmul(
                                pp[:], qt[:, qs], kt[:, kslice],
                                start=(ti == 0), stop=(ti == len(terms) - 1),
                            )
                        nc.scalar.activation(
                            sh[:, kc * 512 : (kc + 1) * 512], pp[:], AF.Copy
                        )
                    # local top-8 + local indices: frees this half's buffer early
                    if _topk_level >= 1:
                        nc.vector.max(m01[:, h * 8 : h * 8 + 8], sh[:])
                    il = kw.tile([P, 8], u16, tag=f"i{h}", name=f"il{h}")
                    if _topk_level >= 2:
                        nc.vector.max_index(il[:], m01[:, h * 8 : h * 8 + 8], sh[:])
                    else:
                        nc.vector.memset(il[:], 0)
                    iloc.append(il)

                # merge by value; resolve indices via a 16-slot position scan
                if _topk_level < 1:
                    nc.vector.memset(m01[:], 0)
                v8 = kw.tile([P, 8], f32, tag="v8")
                nc.vector.max(v8[:], m01[:])
                pos = kw.tile([P, 8], u16, tag="pos")
                nc.vector.max_index(pos[:], v8[:], m01[:])
                ic = kw.tile([P, 16], f32, tag="ic")
                nc.vector.tensor_copy(ic[:, 0:8], iloc[0][:])
                nc.vector.tensor_scalar(
                    ic[:, 8:16], iloc[1][:], float(HALF), None, op0=OP.add
                )
                posf = kw.tile([P, 8], f32, tag="posf")
                nc.vector.tensor_copy(posf[:], pos[:])
                fin = kw.tile([P, 8], f32, tag="fin")
                nc.vector.tensor_copy(fin[:], ic[:, 0:1].to_broadcast([P, 8]))
                mks = kw.tile([P, 8], mybir.dt.uint32, tag="mks")
                for s in range(1, 16):
                    nc.vector.tensor_scalar(
                        mks[:], posf[:], float(s), None, op0=OP.is_equal
                    )
                    nc.vector.copy_predicated(
                        fin[:], mks[:], ic[:, s : s + 1].to_broadcast([P, 8])
                    )
                dma_fin = nc.sync.dma_start(nbrv[qs, :], fin[:])
                fcl = kw.tile([P, 8], f32, tag="fcl")
                nc.vector.tensor_scalar_min(fcl[:], fin[:], float(N - 1))
                fu = kw.tile([P, 8], u16, tag="fu")
                nc.vector.tensor_copy(fu[:], fcl[:])
                dma_u = nc.sync.dma_start(
                    nbr16[qs, :], fu[:]
                )

                if _skip_gather:
                    continue
                # wrapped idx layout for dma_gather: [16, 64] replicated x8
                ix_t = gth.tile([P, 64], u16, tag="ix")
                src = nbr16[qs, :].rearrange("(qh p) k -> p k qh", p=16)
                for gblk in range(8):
                    d = nc.sync.dma_start(ix_t[gblk * 16 : (gblk + 1) * 16, :], src)
                    add_dep_helper(d.ins, dma_u.ins, reason="idx wrap after nbr16 write")

                rr = []
                for gb in range(2):
                    g4 = gth.tile([P, 4, D], f32, tag="g4", name=f"g4_{gb}")
                    gi = nc.gpsimd.dma_gather(
                        g4[:], xr[:],
                        ix_t[:, gb * 32 : (gb + 1) * 32].bitcast(i16),
                        P * 4, P * 4, D,
                    )
                    add_dep_helper(gi.ins, lib_inst.ins, reason="mlp lib before gather")
                    r_t = gth.tile([P, D], f32, tag=f"rt{gb}", name=f"rt{gb}")
                    nc.vector.tensor_reduce(
                        r_t[:],
                        g4[:].rearrange("p k j -> p j k"),
                        axis=mybir.AxisListType.X,
                        op=OP.add,
                    )
                    rr.append(r_t)
                ea = gth.tile([P, D], f32, tag="ea")
                nc.vector.tensor_tensor(ea[:], rr[0][:], rr[1][:], op=OP.add)
                nc.scalar.activation(rr[0][:], ea[:], AF.Copy, scale=0.125)
                nc.sync.dma_start(eattr[qs, :], rr[0][:])

    nc.compile()
    return nc


_CACHED = {}


def _get_kernel(N, D, D2):
    key = (N, D, D2)
    if key not in _CACHED:
        _CACHED[key] = build_kernel(N, D, D2)
    return _CACHED[key]


def _host_fallback_rows(x, W, b, rows, N, K=8):
    """Exact fp32 recompute of top-8 for pathological (tie) rows."""
    h = x.astype(np.float32) @ W.astype(np.float32) + b.astype(np.float32)
    h = np.where(h >= 0, h, np.float32(0.01) * h)
    hn = h / np.maximum(
        np.linalg.norm(h, axis=1, keepdims=True), np.float32(1e-12)
    ).astype(np.float32)
    sim = hn[rows] @ hn.T
    part = np.argpartition(-sim, K, axis=1)[:, : K + 4]
    vals = np.take_along_axis(sim, part, axis=1)
    order = np.argsort(-vals, axis=1, kind="stable")
    return np.take_along_axis(part, order, axis=1)[:, :K]


def kernel(x, W, b):
    x = np.ascontiguousarray(np.asarray(x, dtype=np.float32))
    W = np.ascontiguousarray(np.asarray(W, dtype=np.float32))
    b = np.ascontiguousarray(np.asarray(b, dtype=np.float32))
    N, D = x.shape
    D2 = W.shape[1]
    Q = N // N_CORES
    K = 8

    from concourse.bass_utils import run_bass_kernel_spmd

    nc = _get_kernel(N, D, D2)
    ident = np.eye(P, dtype=np.float32)
    brow = np.ascontiguousarray(np.tile(b.reshape(1, D2), (P, 1)))
    in_maps = []
    for i in range(N_CORES):
        in_maps.append(
            {
                "xr": np.roll(x, -Q * i, axis=0),
                "W": W,
                "b": brow,
                "ident": ident,
            }
        )
    import time as _time

    t0 = _time.time()
    res = run_bass_kernel_spmd(nc, in_maps, core_ids=list(range(N_CORES)))
    kernel.last_exec_wall_s = _time.time() - t0

    nbr = np.empty((N, K), dtype=np.int64)
    edge_attr = np.empty((N, D), dtype=np.float32)
    bad_rows = []
    for i in range(N_CORES):
        r = res.results[i]
        loc = r["nbrv"].astype(np.int64)  # local (rotated) ids, sentinel >= N
        bad = np.unique(np.argwhere(loc >= N)[:, 0])
        glob = (loc + Q * i) % N
        nbr[Q * i : Q * (i + 1)] = glob
        edge_attr[Q * i : Q * (i + 1)] = r["eattr"]
        for rr in bad:
            bad_rows.append(Q * i + int(rr))

    if bad_rows:
        rows = np.array(sorted(set(bad_rows)), dtype=np.int64)
        fixed = _host_fallback_rows(x, W, b, rows, N, K)
        nbr[rows] = fixed
        edge_attr[rows] = x[fixed].mean(axis=1)

    row = nbr.reshape(-1).astype(np.int32)
    col = np.repeat(np.arange(N, dtype=np.int32), K)
    edge_index = np.stack([row, col], axis=0)
    return (x, edge_index, edge_attr)
